# revision 7
# baseline (speedup 1.0000x reference)
"""Multi-head attention forward for Trainium2, 8 NeuronCores.

Problem: B=4, S=2048, D=1024, H=16 heads (dk=64), fp32 reference:
  q/k/v = x @ W{q,k,v}^T + b ; heads split; softmax(q k^T / 8) v ; out @ Wo^T + bo

Sharding: 8 cores = 4 batches x 2 head-groups (8 heads each), Megatron-style:
each core computes its batch's attention for its 8 heads plus the partial
output projection (Wo column slice); host sums the two partials per batch.

Per-core kernel:
  A: V = x@WvT+bv (natural [s,d] layout), QT/KT = (x@W^T)^T per head-pair
     (f32r matmuls, contraction=model dim on partitions, via host-transposed
     x and weights)
  B: per head-pair, per 1024-query-half, per 128-key-tile:
     S^T[k,q] = KT^T@QT      (row-tiled pair of K=64 f32r matmuls)
     P^T = exp(S^T/8)        (ACT, fp16 out, scale fused)
     OT += V-slice^T @ P^T   (col-tiled fp16 matmul pair, M=64 each)
     den += ones32 @ P^T     (4-way col-tiled M=32 fp16 matmuls, one bank)
     then R = recip(den) replicated to OT's partition layout via small
     SBUF->SBUF DMAs, and OT_norm = OT * R (one aligned multiply per chunk)
  C: y_partial = OT_norm^T @ WoT + bo (fp16 matmuls, bo zeros on odd cores)
"""

import sys

sys.path.insert(0, "/opt/trn_rl_repo")

import numpy as np

import concourse.bass as bass  # noqa: F401
import concourse.mybir as mybir
import concourse.tile as tile
from concourse import bacc, bass_utils

B, S, D, H = 4, 2048, 1024, 16
DK = D // H          # 64
G = 2                # head groups (tensor-parallel factor)
DL = D // G          # 512 local features per core
NPAIR = DL // 128    # 4 head-pairs per core
EC = D // 128        # 8 contraction chunks for projections
ST = S // 128        # 16 s-tiles
KT = S // 128        # 16 key tiles
QH = S // 1024       # 2 query halves

F32R = mybir.dt.float32r
F32 = mybir.dt.float32
F16 = mybir.dt.float16

_CACHED = {}


def _build_nc(loop_n=1):
    nc = bacc.Bacc(None, target_bir_lowering=False)

    xT = nc.dram_tensor("xT", [D, S], F32R, kind="ExternalInput")
    wqT = nc.dram_tensor("wqT", [D, DL], F32R, kind="ExternalInput")
    wkT = nc.dram_tensor("wkT", [D, DL], F32R, kind="ExternalInput")
    wvT = nc.dram_tensor("wvT", [D, DL], F32R, kind="ExternalInput")
    woT = nc.dram_tensor("woT", [DL, D], F16, kind="ExternalInput")
    bq = nc.dram_tensor("bq", [DL], F32, kind="ExternalInput")
    bk = nc.dram_tensor("bk", [DL], F32, kind="ExternalInput")
    bv = nc.dram_tensor("bv", [1, DL], F16, kind="ExternalInput")
    bo = nc.dram_tensor("bo", [1, D], F16, kind="ExternalInput")
    y = nc.dram_tensor("y", [S, D], F32, kind="ExternalOutput")

    with tile.TileContext(nc) as tc:
      for _rep in range(loop_n):
        with (
            tc.tile_pool(name="main", bufs=1) as pmain,
            tc.tile_pool(name="qkt", bufs=2) as pqkt,
            tc.tile_pool(name="ptile", bufs=3) as ppt,
            tc.tile_pool(name="rtile", bufs=2) as prt,
            tc.tile_pool(name="ytile", bufs=3) as pyt,
            tc.tile_pool(name="ps512", bufs=1, space="PSUM") as psA,
        ):
            # persistent tiles
            vt = pmain.tile([128, ST, DL], F16, tag="vt")
            ot = pmain.tile([128, NPAIR, S], F16, tag="ot")
            ones32 = pmain.tile([128, 32], F16, tag="ones32")
            ones1h = pmain.tile([1, 128], F16, tag="ones1h")
            bqt = pmain.tile([128, NPAIR], F32, tag="bqt")
            bkt = pmain.tile([128, NPAIR], F32, tag="bkt")
            bvt = pmain.tile([1, DL], F16, tag="bvt")
            bot = pmain.tile([1, D], F16, tag="bot")

            nc.vector.memset(ones32[:], 1.0)
            nc.vector.memset(ones1h[:], 1.0)
            nc.sync.dma_start(bqt[:], bq.ap().rearrange("(p d) -> d p", d=128))
            nc.sync.dma_start(bkt[:], bk.ap().rearrange("(p d) -> d p", d=128))
            nc.sync.dma_start(bvt[:], bv.ap())
            nc.sync.dma_start(bot[:], bo.ap())

            with (
                tc.tile_pool(name="xw", bufs=1) as pxw,
                tc.tile_pool(name="wqk", bufs=2) as pwqk,
                tc.tile_pool(name="psB", bufs=1, space="PSUM") as psB,
                tc.tile_pool(name="psSC", bufs=4, space="PSUM") as psSC,
            ):
                xt = pxw.tile([128, EC, S], F32R, tag="xt")
                wvt = pxw.tile([128, EC, DL], F32R, tag="wvt")
                for ec in range(EC):
                    nc.sync.dma_start(xt[:, ec], xT.ap()[ec * 128:(ec + 1) * 128, :])
                    nc.sync.dma_start(wvt[:, ec], wvT.ap()[ec * 128:(ec + 1) * 128, :])

                # ---- A0: V = x @ WvT + bv, [s, d] natural layout, fp16
                for st in range(ST):
                    vps = psA.tile([128, DL], F32, tag="ps512")
                    for ec in range(EC):
                        nc.tensor.matmul(
                            vps[:], xt[:, ec, st * 128:(st + 1) * 128], wvt[:, ec],
                            start=(ec == 0), stop=False)
                    nc.tensor.matmul(vps[:], ones1h[:], bvt[:], start=False, stop=True)
                    nc.vector.tensor_copy(vt[:, st], vps[:])

                qts = [None] * NPAIR
                kts = [None] * NPAIR

                def phase_a(p):
                    wqp = pwqk.tile([128, EC, 128], F32R, tag="wqp")
                    wkp = pwqk.tile([128, EC, 128], F32R, tag="wkp")
                    for ec in range(EC):
                        nc.sync.dma_start(
                            wqp[:, ec],
                            wqT.ap()[ec * 128:(ec + 1) * 128, p * 128:(p + 1) * 128])
                        nc.sync.dma_start(
                            wkp[:, ec],
                            wkT.ap()[ec * 128:(ec + 1) * 128, p * 128:(p + 1) * 128])
                    qt = pqkt.tile([128, S], F32R, tag="qt")
                    kt = pqkt.tile([128, S], F32R, tag="kt")
                    for dst, wp, bias in ((qt, wqp, bqt), (kt, wkp, bkt)):
                        for qc in range(4):
                            ps = psA.tile([128, 512], F32, tag="ps512")
                            for ec in range(EC):
                                nc.tensor.matmul(
                                    ps[:], wp[:, ec], xt[:, ec, qc * 512:(qc + 1) * 512],
                                    start=(ec == 0), stop=(ec == EC - 1))
                            nc.vector.tensor_scalar_add(
                                dst[:, qc * 512:(qc + 1) * 512], ps[:],
                                bias[:, p:p + 1])
                    qts[p], kts[p] = qt, kt

                def phase_b(p):
                    qt, kt = qts[p], kts[p]
                    for qh in range(QH):
                        otp = psB.tile([128, 1024], F32, tag="otp")
                        dnp = psB.tile([128, 512], F32, tag="dnp")
                        for kti in range(KT):
                            ks = slice(kti * 128, (kti + 1) * 128)
                            p0 = ppt.tile([128, 1024], F16, tag="pt0")
                            p1 = ppt.tile([128, 1024], F16, tag="pt1")
                            scs = []
                            for qc in range(2):
                                q0 = qh * 1024 + qc * 512
                                sc0 = psSC.tile([128, 512], F32, tag="sc")
                                sc1 = psSC.tile([128, 512], F32, tag="sc")
                                nc.tensor.matmul(
                                    sc0[:], kt[0:64, ks], qt[0:64, q0:q0 + 512],
                                    start=True, stop=True, tile_position=(0, 0))
                                nc.tensor.matmul(
                                    sc1[:], kt[64:128, ks], qt[64:128, q0:q0 + 512],
                                    start=True, stop=True, tile_position=(64, 0))
                                scs.append((sc0, sc1))
                            for qc in range(2):
                                cs = slice(qc * 512, (qc + 1) * 512)
                                sc0, sc1 = scs[qc]
                                nc.scalar.activation(
                                    p0[:, cs], sc0[:],
                                    mybir.ActivationFunctionType.Exp, scale=0.125)
                                nc.scalar.activation(
                                    p1[:, cs], sc1[:],
                                    mybir.ActivationFunctionType.Exp, scale=0.125)
                            first, last = kti == 0, kti == KT - 1
                            for qc in range(2):
                                cs = slice(qc * 512, (qc + 1) * 512)
                                nc.tensor.matmul(
                                    otp[0:64, cs],
                                    vt[:, kti, p * 128:p * 128 + 64], p0[:, cs],
                                    start=first, stop=last, tile_position=(0, 0))
                                nc.tensor.matmul(
                                    otp[64:128, cs],
                                    vt[:, kti, p * 128 + 64:(p + 1) * 128], p1[:, cs],
                                    start=first, stop=last, tile_position=(0, 64))
                            # denominators: 4-way col-tiled M=32 into one bank
                            # rows [0:32]=h0/qc0 [32:64]=h0/qc1 [64:96]=h1/qc0
                            # [96:128]=h1/qc1, each denominator replicated x32
                            nc.tensor.matmul(
                                dnp[0:32, :], ones32[:, :], p0[:, 0:512],
                                start=first, stop=last, tile_position=(0, 0))
                            nc.tensor.matmul(
                                dnp[32:64, :], ones32[:, :], p0[:, 512:1024],
                                start=first, stop=last, tile_position=(0, 32))
                            nc.tensor.matmul(
                                dnp[64:96, :], ones32[:, :], p1[:, 0:512],
                                start=first, stop=last, tile_position=(0, 64))
                            nc.tensor.matmul(
                                dnp[96:128, :], ones32[:, :], p1[:, 512:1024],
                                start=first, stop=last, tile_position=(0, 96))
                        r0 = prt.tile([128, 512], F32, tag="r0")
                        rt = prt.tile([128, 1024], F32, tag="rt")
                        nc.vector.reciprocal(r0[:], dnp[:])
                        # replicate each 32-row denominator block to the 64-row
                        # (head) x 512-col (q chunk) layout OT uses
                        for qc in range(2):
                            cs = slice(qc * 512, (qc + 1) * 512)
                            src0 = r0[qc * 32:(qc + 1) * 32, :]          # h0
                            src1 = r0[64 + qc * 32:64 + (qc + 1) * 32, :]  # h1
                            nc.sync.dma_start(rt[0:32, cs], src0)
                            nc.sync.dma_start(rt[32:64, cs], src0)
                            nc.sync.dma_start(rt[64:96, cs], src1)
                            nc.sync.dma_start(rt[96:128, cs], src1)
                        for qc in range(2):
                            cs = slice(qc * 512, (qc + 1) * 512)
                            qabs = slice(qh * 1024 + qc * 512,
                                         qh * 1024 + (qc + 1) * 512)
                            nc.vector.tensor_mul(ot[:, p, qabs], otp[:, cs], rt[:, cs])

                for p in range(NPAIR):
                    phase_a(p)
                    if p > 0:
                        phase_b(p - 1)
                phase_b(NPAIR - 1)

            # ---- C: y = OT^T @ WoT + bo
            with (
                tc.tile_pool(name="wo", bufs=1) as pwo,
                tc.tile_pool(name="psC", bufs=4, space="PSUM") as psC,
            ):
                wot = pwo.tile([128, NPAIR, D], F16, tag="wot")
                for dc in range(NPAIR):
                    nc.sync.dma_start(
                        wot[:, dc], woT.ap()[dc * 128:(dc + 1) * 128, :])
                for st in range(ST):
                    ss = slice(st * 128, (st + 1) * 128)
                    for e2 in range(2):
                        es = slice(e2 * 512, (e2 + 1) * 512)
                        yps = psC.tile([128, 512], F32, tag="yps")
                        for dc in range(NPAIR):
                            nc.tensor.matmul(
                                yps[:], ot[:, dc, ss], wot[:, dc, es],
                                start=(dc == 0), stop=False)
                        nc.tensor.matmul(
                            yps[:], ones1h[:], bot[:, es], start=False, stop=True)
                        yt = pyt.tile([128, 512], F32, tag="yt")
                        nc.vector.tensor_copy(yt[:], yps[:])
                        nc.sync.dma_start(y.ap()[ss, es], yt[:])

    nc.compile()
    return nc


def _get_nc(loop_n=1):
    key = f"nc{loop_n}"
    if key not in _CACHED:
        _CACHED[key] = _build_nc(loop_n)
    return _CACHED[key]


def kernel(encoder_input, attention_mask, Wq_w, Wq_b, Wk_w, Wk_b, Wv_w, Wv_b,
           Wo_w, Wo_b):
    del attention_mask  # dead input in the reference forward
    encoder_input = np.asarray(encoder_input, dtype=np.float32)
    Wq_w = np.asarray(Wq_w, dtype=np.float32)
    Wk_w = np.asarray(Wk_w, dtype=np.float32)
    Wv_w = np.asarray(Wv_w, dtype=np.float32)
    Wo_w = np.asarray(Wo_w, dtype=np.float32)
    Wq_b = np.asarray(Wq_b, dtype=np.float32)
    Wk_b = np.asarray(Wk_b, dtype=np.float32)
    Wv_b = np.asarray(Wv_b, dtype=np.float32)
    Wo_b = np.asarray(Wo_b, dtype=np.float32)

    nc = _get_nc()

    in_maps = []
    for core in range(8):
        b, g = divmod(core, G)
        gs = slice(g * DL, (g + 1) * DL)
        in_maps.append({
            "xT": np.ascontiguousarray(encoder_input[b].T),
            "wqT": np.ascontiguousarray(Wq_w[gs, :].T),
            "wkT": np.ascontiguousarray(Wk_w[gs, :].T),
            "wvT": np.ascontiguousarray(Wv_w[gs, :].T),
            "woT": np.ascontiguousarray(Wo_w[:, gs].T).astype(np.float16),
            "bq": np.ascontiguousarray(Wq_b[gs]),
            "bk": np.ascontiguousarray(Wk_b[gs]),
            "bv": Wv_b[gs].astype(np.float16).reshape(1, DL),
            "bo": (Wo_b if g == 0 else np.zeros_like(Wo_b))
                  .astype(np.float16).reshape(1, D),
        })

    res = bass_utils.run_bass_kernel_spmd(nc, in_maps, core_ids=list(range(8)))
    out = np.empty((B, S, D), dtype=np.float32)
    for b in range(B):
        out[b] = res.results[G * b]["y"] + res.results[G * b + 1]["y"]
    return out


# revision 13
# speedup vs baseline: 1.1269x; 1.1269x over previous
"""Multi-head attention forward for Trainium2, 8 NeuronCores.

Problem: B=4, S=2048, D=1024, H=16 heads (dk=64), fp32 reference:
  q/k/v = x @ W{q,k,v}^T + b ; heads split; softmax(q k^T / 8) v ; out @ Wo^T + bo

Sharding: 8 cores = 4 batches x 2 head-groups (8 heads each), Megatron-style:
each core computes its batch's attention for its 8 heads plus the partial
output projection (Wo column slice); host sums the two partials per batch.

Per-core kernel:
  A: V = x@WvT+bv (natural [s,d] layout), QT/KT = (x@W^T)^T per head-pair
     (f32r matmuls, contraction=model dim on partitions, via host-transposed
     x and weights)
  B: per head-pair, per 1024-query-half, per 128-key-tile:
     S^T[k,q] = KT^T@QT      (row-tiled pair of K=64 f32r matmuls)
     P^T = exp(S^T/8)        (ACT, fp16 out, scale fused)
     OT += V-slice^T @ P^T   (col-tiled fp16 matmul pair, M=64 each)
     den += ones32 @ P^T     (4-way col-tiled M=32 fp16 matmuls, one bank)
     then R = recip(den) replicated to OT's partition layout via small
     SBUF->SBUF DMAs, and OT_norm = OT * R (one aligned multiply per chunk)
  C: y_partial = OT_norm^T @ WoT + bo (fp16 matmuls, bo zeros on odd cores)
"""

import sys

sys.path.insert(0, "/opt/trn_rl_repo")

import numpy as np

import concourse.bass as bass  # noqa: F401
import concourse.mybir as mybir
import concourse.tile as tile
from concourse import bacc, bass_utils

B, S, D, H = 4, 2048, 1024, 16
DK = D // H          # 64
G = 2                # head groups (tensor-parallel factor)
DL = D // G          # 512 local features per core
NPAIR = DL // 128    # 4 head-pairs per core
EC = D // 128        # 8 contraction chunks for projections
ST = S // 128        # 16 s-tiles
KT = S // 128        # 16 key tiles
QH = S // 1024       # 2 query halves

F32R = mybir.dt.float32r
F32 = mybir.dt.float32
F16 = mybir.dt.float16

_CACHED = {}


def _build_nc(loop_n=1):
    nc = bacc.Bacc(None, target_bir_lowering=False)

    xT = nc.dram_tensor("xT", [D, S], F32R, kind="ExternalInput")
    wqT = nc.dram_tensor("wqT", [D, DL], F32R, kind="ExternalInput")
    wkT = nc.dram_tensor("wkT", [D, DL], F32R, kind="ExternalInput")
    wvT = nc.dram_tensor("wvT", [D, DL], F32R, kind="ExternalInput")
    woT = nc.dram_tensor("woT", [DL, D], F16, kind="ExternalInput")
    bq = nc.dram_tensor("bq", [DL], F32, kind="ExternalInput")
    bk = nc.dram_tensor("bk", [DL], F32, kind="ExternalInput")
    bv = nc.dram_tensor("bv", [1, DL], F16, kind="ExternalInput")
    bo = nc.dram_tensor("bo", [1, D], F16, kind="ExternalInput")
    y = nc.dram_tensor("y", [S, D], F32, kind="ExternalOutput")

    with tile.TileContext(nc) as tc:
      for _rep in range(loop_n):
        with (
            tc.tile_pool(name="main", bufs=1) as pmain,
            tc.tile_pool(name="qkt", bufs=2) as pqkt,
            tc.tile_pool(name="ptile", bufs=3) as ppt,
            tc.tile_pool(name="rtile", bufs=2) as prt,
            tc.tile_pool(name="ytile", bufs=3) as pyt,
            tc.tile_pool(name="ps512", bufs=1, space="PSUM") as psA,
        ):
            # persistent tiles
            vt = pmain.tile([128, ST, DL], F16, tag="vt")
            ot = pmain.tile([128, NPAIR, S], F16, tag="ot")
            ones32 = pmain.tile([128, 32], F16, tag="ones32")
            ones1h = pmain.tile([1, 128], F16, tag="ones1h")
            bqt = pmain.tile([128, NPAIR], F32, tag="bqt")
            bkt = pmain.tile([128, NPAIR], F32, tag="bkt")
            bvt = pmain.tile([1, DL], F16, tag="bvt")
            bot = pmain.tile([1, D], F16, tag="bot")

            nc.vector.memset(ones32[:], 1.0)
            nc.vector.memset(ones1h[:], 1.0)
            nc.sync.dma_start(bqt[:], bq.ap().rearrange("(p d) -> d p", d=128))
            nc.sync.dma_start(bkt[:], bk.ap().rearrange("(p d) -> d p", d=128))
            nc.sync.dma_start(bvt[:], bv.ap())
            nc.sync.dma_start(bot[:], bo.ap())

            with (
                tc.tile_pool(name="xw", bufs=1) as pxw,
                tc.tile_pool(name="wqk", bufs=2) as pwqk,
                tc.tile_pool(name="psB", bufs=1, space="PSUM") as psB,
            ):
                xt = pxw.tile([128, EC, S], F32R, tag="xt")
                wvt = pxw.tile([128, EC, DL], F32R, tag="wvt")
                for ec in range(EC):
                    nc.sync.dma_start(xt[:, ec], xT.ap()[ec * 128:(ec + 1) * 128, :])
                    nc.sync.dma_start(wvt[:, ec], wvT.ap()[ec * 128:(ec + 1) * 128, :])

                # ---- A0: V = x @ WvT + bv, [s, d] natural layout, fp16
                for st in range(ST):
                    vps = psA.tile([128, DL], F32, tag="ps512")
                    for ec in range(EC):
                        nc.tensor.matmul(
                            vps[:], xt[:, ec, st * 128:(st + 1) * 128], wvt[:, ec],
                            start=(ec == 0), stop=False)
                    nc.tensor.matmul(vps[:], ones1h[:], bvt[:], start=False, stop=True)
                    nc.vector.tensor_copy(vt[:, st], vps[:])

                qts = [None] * NPAIR
                kts = [None] * NPAIR

                def phase_a(p):
                    wqp = pwqk.tile([128, EC, 128], F32R, tag="wqp")
                    wkp = pwqk.tile([128, EC, 128], F32R, tag="wkp")
                    for ec in range(EC):
                        nc.sync.dma_start(
                            wqp[:, ec],
                            wqT.ap()[ec * 128:(ec + 1) * 128, p * 128:(p + 1) * 128])
                        nc.sync.dma_start(
                            wkp[:, ec],
                            wkT.ap()[ec * 128:(ec + 1) * 128, p * 128:(p + 1) * 128])
                    qt = pqkt.tile([128, S], F32R, tag="qt")
                    kt = pqkt.tile([128, S], F32R, tag="kt")
                    for dst, wp, bias in ((qt, wqp, bqt), (kt, wkp, bkt)):
                        for qc in range(4):
                            ps = psA.tile([128, 512], F32, tag="ps512")
                            for ec in range(EC):
                                nc.tensor.matmul(
                                    ps[:], wp[:, ec], xt[:, ec, qc * 512:(qc + 1) * 512],
                                    start=(ec == 0), stop=(ec == EC - 1))
                            nc.vector.tensor_scalar_add(
                                dst[:, qc * 512:(qc + 1) * 512], ps[:],
                                bias[:, p:p + 1])
                    qts[p], kts[p] = qt, kt

                def phase_b(p):
                    qt, kt = qts[p], kts[p]
                    for qh in range(QH):
                        st0 = psB.tile([128, 1024], F32, tag="st0")
                        st1 = psB.tile([128, 1024], F32, tag="st1")
                        otp = psB.tile([128, 1024], F32, tag="otp")
                        dnp = psB.tile([128, 512], F32, tag="dnp")
                        for kti in range(KT):
                            ks = slice(kti * 128, (kti + 1) * 128)
                            p0 = ppt.tile([128, 1024], F16, tag="pt0")
                            p1 = ppt.tile([128, 1024], F16, tag="pt1")
                            for qc in range(2):
                                q0 = qh * 1024 + qc * 512
                                cs = slice(qc * 512, (qc + 1) * 512)
                                nc.tensor.matmul(
                                    st0[:, cs], kt[0:64, ks], qt[0:64, q0:q0 + 512],
                                    start=True, stop=True, tile_position=(0, 0))
                                nc.tensor.matmul(
                                    st1[:, cs], kt[64:128, ks], qt[64:128, q0:q0 + 512],
                                    start=True, stop=True, tile_position=(64, 0))
                            nc.scalar.activation(
                                p0[:], st0[:], mybir.ActivationFunctionType.Exp,
                                scale=0.125)
                            nc.scalar.activation(
                                p1[:], st1[:], mybir.ActivationFunctionType.Exp,
                                scale=0.125)
                            first, last = kti == 0, kti == KT - 1
                            for qc in range(2):
                                cs = slice(qc * 512, (qc + 1) * 512)
                                nc.tensor.matmul(
                                    otp[0:64, cs],
                                    vt[:, kti, p * 128:p * 128 + 64], p0[:, cs],
                                    start=first, stop=last, tile_position=(0, 0))
                                nc.tensor.matmul(
                                    otp[64:128, cs],
                                    vt[:, kti, p * 128 + 64:(p + 1) * 128], p1[:, cs],
                                    start=first, stop=last, tile_position=(0, 64))
                            nc.tensor.matmul(
                                dnp[0:32, :], ones32[:, :], p0[:, 0:512],
                                start=first, stop=last, tile_position=(0, 0))
                            nc.tensor.matmul(
                                dnp[32:64, :], ones32[:, :], p0[:, 512:1024],
                                start=first, stop=last, tile_position=(0, 32))
                            nc.tensor.matmul(
                                dnp[64:96, :], ones32[:, :], p1[:, 0:512],
                                start=first, stop=last, tile_position=(0, 64))
                            nc.tensor.matmul(
                                dnp[96:128, :], ones32[:, :], p1[:, 512:1024],
                                start=first, stop=last, tile_position=(0, 96))
                        r0 = prt.tile([128, 512], F32, tag="r0")
                        rt = prt.tile([128, 1024], F32, tag="rt")
                        nc.vector.reciprocal(r0[:], dnp[:])
                        # replicate each 32-row denominator block to the 64-row
                        # (head) x 512-col (q chunk) layout OT uses
                        for qc in range(2):
                            cs = slice(qc * 512, (qc + 1) * 512)
                            src0 = r0[qc * 32:(qc + 1) * 32, :]          # h0
                            src1 = r0[64 + qc * 32:64 + (qc + 1) * 32, :]  # h1
                            nc.sync.dma_start(rt[0:32, cs], src0)
                            nc.sync.dma_start(rt[32:64, cs], src0)
                            nc.sync.dma_start(rt[64:96, cs], src1)
                            nc.sync.dma_start(rt[96:128, cs], src1)
                        for qc in range(2):
                            cs = slice(qc * 512, (qc + 1) * 512)
                            qabs = slice(qh * 1024 + qc * 512,
                                         qh * 1024 + (qc + 1) * 512)
                            nc.vector.tensor_mul(ot[:, p, qabs], otp[:, cs], rt[:, cs])

                for p in range(NPAIR):
                    phase_a(p)
                    if p > 0:
                        phase_b(p - 1)
                phase_b(NPAIR - 1)

            # ---- C: y = OT^T @ WoT + bo
            with (
                tc.tile_pool(name="wo", bufs=1) as pwo,
                tc.tile_pool(name="psC", bufs=4, space="PSUM") as psC,
            ):
                wot = pwo.tile([128, NPAIR, D], F16, tag="wot")
                for dc in range(NPAIR):
                    nc.sync.dma_start(
                        wot[:, dc], woT.ap()[dc * 128:(dc + 1) * 128, :])
                for st in range(ST):
                    ss = slice(st * 128, (st + 1) * 128)
                    for e2 in range(2):
                        es = slice(e2 * 512, (e2 + 1) * 512)
                        yps = psC.tile([128, 512], F32, tag="yps")
                        for dc in range(NPAIR):
                            nc.tensor.matmul(
                                yps[:], ot[:, dc, ss], wot[:, dc, es],
                                start=(dc == 0), stop=False)
                        nc.tensor.matmul(
                            yps[:], ones1h[:], bot[:, es], start=False, stop=True)
                        yt = pyt.tile([128, 512], F32, tag="yt")
                        nc.vector.tensor_copy(yt[:], yps[:])
                        nc.sync.dma_start(y.ap()[ss, es], yt[:])

    nc.compile()
    return nc


def _get_nc(loop_n=1):
    key = f"nc{loop_n}"
    if key not in _CACHED:
        _CACHED[key] = _build_nc(loop_n)
    return _CACHED[key]


def _get_runner():
    """Build the 8-core SPMD executable once and cache it, so repeated
    kernel() calls skip jax re-tracing and NEFF compilation."""
    if "runner" in _CACHED:
        return _CACHED["runner"]

    import jax
    from jax.sharding import Mesh, NamedSharding, PartitionSpec
    from jax.experimental.shard_map import shard_map
    from concourse import bass2jax
    from concourse.bass2jax import _bass_exec_p, install_neuronx_cc_hook

    nc = _get_nc()
    install_neuronx_cc_hook()
    partition_name = nc.partition_id_tensor.name if nc.partition_id_tensor else None
    in_names, out_names, out_avals, zero_outs = [], [], [], []
    for alloc in nc.m.functions[0].allocations:
        if not isinstance(alloc, mybir.MemoryLocationSet):
            continue
        name = alloc.memorylocations[0].name
        if alloc.kind == "ExternalInput":
            if name != partition_name:
                in_names.append(name)
        elif alloc.kind == "ExternalOutput":
            out_names.append(name)
            shape = tuple(alloc.tensor_shape)
            dtype = mybir.dt.np(alloc.dtype)
            out_avals.append(jax.core.ShapedArray(shape, dtype))
            zero_outs.append(np.zeros(shape, dtype))
    n_params, n_outs = len(in_names), len(out_avals)
    all_names = in_names + out_names + ([partition_name] if partition_name else [])

    def _body(*args):
        operands = list(args)
        if partition_name is not None:
            operands.append(bass2jax.partition_id_tensor())
        outs = _bass_exec_p.bind(
            *operands,
            out_avals=tuple(out_avals),
            in_names=tuple(all_names),
            out_names=tuple(out_names),
            lowering_input_output_aliases=(),
            sim_require_finite=True,
            sim_require_nnan=True,
            nc=nc,
        )
        return tuple(outs)

    devices = jax.devices()[:8]
    mesh = Mesh(np.asarray(devices), ("core",))
    f = jax.jit(
        shard_map(
            _body, mesh=mesh,
            in_specs=(PartitionSpec("core"),) * (n_params + n_outs),
            out_specs=(PartitionSpec("core"),) * n_outs,
            check_rep=False,
        ),
        donate_argnums=tuple(range(n_params, n_params + n_outs)),
        keep_unused=True,
    )
    shard = NamedSharding(mesh, PartitionSpec("core"))
    state = {
        "f": f, "in_names": in_names, "out_names": out_names,
        "zero_outs": zero_outs, "shard": shard, "jax": jax, "last_outs": None,
    }
    _CACHED["runner"] = state
    return state


def kernel(encoder_input, attention_mask, Wq_w, Wq_b, Wk_w, Wk_b, Wv_w, Wv_b,
           Wo_w, Wo_b):
    del attention_mask  # dead input in the reference forward
    encoder_input = np.asarray(encoder_input, dtype=np.float32)
    Wq_w = np.asarray(Wq_w, dtype=np.float32)
    Wk_w = np.asarray(Wk_w, dtype=np.float32)
    Wv_w = np.asarray(Wv_w, dtype=np.float32)
    Wo_w = np.asarray(Wo_w, dtype=np.float32)
    Wq_b = np.asarray(Wq_b, dtype=np.float32)
    Wk_b = np.asarray(Wk_b, dtype=np.float32)
    Wv_b = np.asarray(Wv_b, dtype=np.float32)
    Wo_b = np.asarray(Wo_b, dtype=np.float32)

    r = _get_runner()
    jax = r["jax"]

    woT_f16 = {}
    in_maps = []
    for core in range(8):
        b, g = divmod(core, G)
        gs = slice(g * DL, (g + 1) * DL)
        if g not in woT_f16:
            woT_f16[g] = np.ascontiguousarray(Wo_w[:, gs].T).astype(np.float16)
        in_maps.append({
            "xT": np.ascontiguousarray(encoder_input[b].T),
            "wqT": np.ascontiguousarray(Wq_w[gs, :].T),
            "wkT": np.ascontiguousarray(Wk_w[gs, :].T),
            "wvT": np.ascontiguousarray(Wv_w[gs, :].T),
            "woT": woT_f16[g],
            "bq": np.ascontiguousarray(Wq_b[gs]),
            "bk": np.ascontiguousarray(Wk_b[gs]),
            "bv": Wv_b[gs].astype(np.float16).reshape(1, DL),
            "bo": (Wo_b if g == 0 else np.zeros_like(Wo_b))
                  .astype(np.float16).reshape(1, D),
        })

    concat_in = [
        jax.device_put(
            np.concatenate([in_maps[c][n] for c in range(8)], axis=0), r["shard"])
        for n in r["in_names"]
    ]
    outs = r["last_outs"]
    if outs is None:
        outs = [
            jax.device_put(
                np.zeros((8 * z.shape[0], *z.shape[1:]), z.dtype), r["shard"])
            for z in r["zero_outs"]
        ]
    outs = r["f"](*concat_in, *outs)
    np_outs = [np.asarray(o) for o in outs]
    # keep the returned device buffers to donate on the next call
    r["last_outs"] = list(outs)

    per_core = {}
    for i, nme in enumerate(r["out_names"]):
        full = np_outs[i].reshape(8, -1, *np_outs[i].shape[1:])
        per_core[nme] = full

    y = per_core["y"]
    out = np.empty((B, S, D), dtype=np.float32)
    for b in range(B):
        out[b] = y[G * b] + y[G * b + 1]
    return out


# revision 15
# speedup vs baseline: 1.1550x; 1.0249x over previous
"""Multi-head attention forward for Trainium2, 8 NeuronCores.

Problem: B=4, S=2048, D=1024, H=16 heads (dk=64), fp32 reference:
  q/k/v = x @ W{q,k,v}^T + b ; heads split; softmax(q k^T / 8) v ; out @ Wo^T + bo

Sharding: 8 cores = 4 batches x 2 head-groups (8 heads each), Megatron-style:
each core computes its batch's attention for its 8 heads plus the partial
output projection (Wo column slice); host sums the two partials per batch.

Per-core kernel:
  A: V = x@WvT+bv (natural [s,d] layout), QT/KT = (x@W^T)^T per head-pair
     (f32r matmuls, contraction=model dim on partitions, via host-transposed
     x and weights)
  B: per head-pair, per 1024-query-half, per 128-key-tile:
     S^T[k,q] = KT^T@QT      (row-tiled pair of K=64 f32r matmuls)
     P^T = exp(S^T/8)        (ACT, fp16 out, scale fused)
     OT += V-slice^T @ P^T   (col-tiled fp16 matmul pair, M=64 each)
     den += ones32 @ P^T     (4-way col-tiled M=32 fp16 matmuls, one bank)
     then R = recip(den) replicated to OT's partition layout via small
     SBUF->SBUF DMAs, and OT_norm = OT * R (one aligned multiply per chunk)
  C: y_partial = OT_norm^T @ WoT + bo (fp16 matmuls, bo zeros on odd cores)
"""

import sys

sys.path.insert(0, "/opt/trn_rl_repo")

import numpy as np

import concourse.bass as bass  # noqa: F401
import concourse.mybir as mybir
import concourse.tile as tile
from concourse import bacc, bass_utils

B, S, D, H = 4, 2048, 1024, 16
DK = D // H          # 64
G = 2                # head groups (tensor-parallel factor)
DL = D // G          # 512 local features per core
NPAIR = DL // 128    # 4 head-pairs per core
EC = D // 128        # 8 contraction chunks for projections
ST = S // 128        # 16 s-tiles
KT = S // 128        # 16 key tiles
QH = S // 1024       # 2 query halves

F32R = mybir.dt.float32r
F32 = mybir.dt.float32
F16 = mybir.dt.float16

_CACHED = {}


def _build_nc(loop_n=1):
    nc = bacc.Bacc(None, target_bir_lowering=False)

    xT = nc.dram_tensor("xT", [D, S], F32R, kind="ExternalInput")
    wqT = nc.dram_tensor("wqT", [D, DL], F32R, kind="ExternalInput")
    wkT = nc.dram_tensor("wkT", [D, DL], F32R, kind="ExternalInput")
    wvT = nc.dram_tensor("wvT", [D, DL], F32R, kind="ExternalInput")
    woT = nc.dram_tensor("woT", [DL, D], F16, kind="ExternalInput")
    bq = nc.dram_tensor("bq", [DL], F32, kind="ExternalInput")
    bk = nc.dram_tensor("bk", [DL], F32, kind="ExternalInput")
    bv = nc.dram_tensor("bv", [1, DL], F16, kind="ExternalInput")
    bo = nc.dram_tensor("bo", [1, D], F16, kind="ExternalInput")
    y = nc.dram_tensor("y", [S, D], F32, kind="ExternalOutput")

    with tile.TileContext(nc) as tc:
      for _rep in range(loop_n):
        with (
            tc.tile_pool(name="main", bufs=1) as pmain,
            tc.tile_pool(name="qkt", bufs=2) as pqkt,
            tc.tile_pool(name="ptile", bufs=3) as ppt,
            tc.tile_pool(name="rtile", bufs=2) as prt,
            tc.tile_pool(name="ytile", bufs=3) as pyt,
            tc.tile_pool(name="ps512", bufs=1, space="PSUM") as psA,
        ):
            # persistent tiles
            vt = pmain.tile([128, ST, DL], F16, tag="vt")
            ot = pmain.tile([128, NPAIR, S], F16, tag="ot")
            ones32 = pmain.tile([128, 32], F16, tag="ones32")
            ones1h = pmain.tile([1, 128], F16, tag="ones1h")
            bqt = pmain.tile([128, NPAIR], F32, tag="bqt")
            bkt = pmain.tile([128, NPAIR], F32, tag="bkt")
            bvt = pmain.tile([1, DL], F16, tag="bvt")
            bot = pmain.tile([1, D], F16, tag="bot")

            nc.vector.memset(ones32[:], 1.0)
            nc.vector.memset(ones1h[:], 1.0)
            nc.sync.dma_start(bqt[:], bq.ap().rearrange("(p d) -> d p", d=128))
            nc.sync.dma_start(bkt[:], bk.ap().rearrange("(p d) -> d p", d=128))
            nc.sync.dma_start(bvt[:], bv.ap())
            nc.sync.dma_start(bot[:], bo.ap())

            with (
                tc.tile_pool(name="xw", bufs=1) as pxw,
                tc.tile_pool(name="wqk", bufs=2) as pwqk,
                tc.tile_pool(name="psB", bufs=1, space="PSUM") as psB,
            ):
                xt = pxw.tile([128, EC, S], F32R, tag="xt")
                wvt = pxw.tile([128, EC, DL], F32R, tag="wvt")
                for ec in range(EC):
                    nc.sync.dma_start(xt[:, ec], xT.ap()[ec * 128:(ec + 1) * 128, :])
                    nc.sync.dma_start(wvt[:, ec], wvT.ap()[ec * 128:(ec + 1) * 128, :])

                def phase_v():
                    # V = x @ WvT + bv, [s, d] natural layout, fp16
                    for st in range(ST):
                        vps = psA.tile([128, DL], F32, tag="ps512")
                        for ec in range(EC):
                            nc.tensor.matmul(
                                vps[:], xt[:, ec, st * 128:(st + 1) * 128], wvt[:, ec],
                                start=(ec == 0), stop=False)
                        nc.tensor.matmul(vps[:], ones1h[:], bvt[:],
                                         start=False, stop=True)
                        nc.vector.tensor_copy(vt[:, st], vps[:])

                qts = [None] * NPAIR
                kts = [None] * NPAIR

                def phase_a(p):
                    wqp = pwqk.tile([128, EC, 128], F32R, tag="wqp")
                    wkp = pwqk.tile([128, EC, 128], F32R, tag="wkp")
                    for ec in range(EC):
                        nc.sync.dma_start(
                            wqp[:, ec],
                            wqT.ap()[ec * 128:(ec + 1) * 128, p * 128:(p + 1) * 128])
                        nc.sync.dma_start(
                            wkp[:, ec],
                            wkT.ap()[ec * 128:(ec + 1) * 128, p * 128:(p + 1) * 128])
                    qt = pqkt.tile([128, S], F32R, tag="qt")
                    kt = pqkt.tile([128, S], F32R, tag="kt")
                    for dst, wp, bias in ((qt, wqp, bqt), (kt, wkp, bkt)):
                        for qc in range(4):
                            ps = psA.tile([128, 512], F32, tag="ps512")
                            for ec in range(EC):
                                nc.tensor.matmul(
                                    ps[:], wp[:, ec], xt[:, ec, qc * 512:(qc + 1) * 512],
                                    start=(ec == 0), stop=(ec == EC - 1))
                            nc.vector.tensor_scalar_add(
                                dst[:, qc * 512:(qc + 1) * 512], ps[:],
                                bias[:, p:p + 1])
                    qts[p], kts[p] = qt, kt

                def phase_b(p):
                    qt, kt = qts[p], kts[p]
                    for qh in range(QH):
                        st0 = psB.tile([128, 1024], F32, tag="st0")
                        st1 = psB.tile([128, 1024], F32, tag="st1")
                        otp = psB.tile([128, 1024], F32, tag="otp")
                        dnp = psB.tile([128, 512], F32, tag="dnp")
                        for kti in range(KT):
                            ks = slice(kti * 128, (kti + 1) * 128)
                            p0 = ppt.tile([128, 1024], F16, tag="pt0")
                            p1 = ppt.tile([128, 1024], F16, tag="pt1")
                            for qc in range(2):
                                q0 = qh * 1024 + qc * 512
                                cs = slice(qc * 512, (qc + 1) * 512)
                                nc.tensor.matmul(
                                    st0[:, cs], kt[0:64, ks], qt[0:64, q0:q0 + 512],
                                    start=True, stop=True, tile_position=(0, 0))
                                nc.tensor.matmul(
                                    st1[:, cs], kt[64:128, ks], qt[64:128, q0:q0 + 512],
                                    start=True, stop=True, tile_position=(64, 0))
                            nc.scalar.activation(
                                p0[:], st0[:], mybir.ActivationFunctionType.Exp,
                                scale=0.125)
                            nc.scalar.activation(
                                p1[:], st1[:], mybir.ActivationFunctionType.Exp,
                                scale=0.125)
                            first, last = kti == 0, kti == KT - 1
                            for qc in range(2):
                                cs = slice(qc * 512, (qc + 1) * 512)
                                nc.tensor.matmul(
                                    otp[0:64, cs],
                                    vt[:, kti, p * 128:p * 128 + 64], p0[:, cs],
                                    start=first, stop=last, tile_position=(0, 0))
                                nc.tensor.matmul(
                                    otp[64:128, cs],
                                    vt[:, kti, p * 128 + 64:(p + 1) * 128], p1[:, cs],
                                    start=first, stop=last, tile_position=(0, 64))
                            nc.tensor.matmul(
                                dnp[0:32, :], ones32[:, :], p0[:, 0:512],
                                start=first, stop=last, tile_position=(0, 0))
                            nc.tensor.matmul(
                                dnp[32:64, :], ones32[:, :], p0[:, 512:1024],
                                start=first, stop=last, tile_position=(0, 32))
                            nc.tensor.matmul(
                                dnp[64:96, :], ones32[:, :], p1[:, 0:512],
                                start=first, stop=last, tile_position=(0, 64))
                            nc.tensor.matmul(
                                dnp[96:128, :], ones32[:, :], p1[:, 512:1024],
                                start=first, stop=last, tile_position=(0, 96))
                        r0 = prt.tile([128, 512], F32, tag="r0")
                        rt = prt.tile([128, 1024], F32, tag="rt")
                        nc.vector.reciprocal(r0[:], dnp[:])
                        # replicate each 32-row denominator block to the 64-row
                        # (head) x 512-col (q chunk) layout OT uses
                        for qc in range(2):
                            cs = slice(qc * 512, (qc + 1) * 512)
                            src0 = r0[qc * 32:(qc + 1) * 32, :]          # h0
                            src1 = r0[64 + qc * 32:64 + (qc + 1) * 32, :]  # h1
                            nc.sync.dma_start(rt[0:32, cs], src0)
                            nc.sync.dma_start(rt[32:64, cs], src0)
                            nc.sync.dma_start(rt[64:96, cs], src1)
                            nc.sync.dma_start(rt[96:128, cs], src1)
                        for qc in range(2):
                            cs = slice(qc * 512, (qc + 1) * 512)
                            qabs = slice(qh * 1024 + qc * 512,
                                         qh * 1024 + (qc + 1) * 512)
                            nc.vector.tensor_mul(ot[:, p, qabs], otp[:, cs], rt[:, cs])

                phase_v()
                for p in range(NPAIR):
                    phase_a(p)
                    if p > 0:
                        phase_b(p - 1)
                phase_b(NPAIR - 1)

            # ---- C: y = OT^T @ WoT + bo
            with (
                tc.tile_pool(name="wo", bufs=1) as pwo,
                tc.tile_pool(name="psC", bufs=4, space="PSUM") as psC,
            ):
                wot = pwo.tile([128, NPAIR, D], F16, tag="wot")
                for dc in range(NPAIR):
                    nc.sync.dma_start(
                        wot[:, dc], woT.ap()[dc * 128:(dc + 1) * 128, :])
                for st in range(ST):
                    ss = slice(st * 128, (st + 1) * 128)
                    for e2 in range(2):
                        es = slice(e2 * 512, (e2 + 1) * 512)
                        yps = psC.tile([128, 512], F32, tag="yps")
                        for dc in range(NPAIR):
                            nc.tensor.matmul(
                                yps[:], ot[:, dc, ss], wot[:, dc, es],
                                start=(dc == 0), stop=False)
                        nc.tensor.matmul(
                            yps[:], ones1h[:], bot[:, es], start=False, stop=True)
                        yt = pyt.tile([128, 512], F32, tag="yt")
                        nc.vector.tensor_copy(yt[:], yps[:])
                        nc.sync.dma_start(y.ap()[ss, es], yt[:])

    nc.compile()
    return nc


def _get_nc(loop_n=1):
    key = f"nc{loop_n}"
    if key not in _CACHED:
        _CACHED[key] = _build_nc(loop_n)
    return _CACHED[key]


def _get_runner():
    """Build the 8-core SPMD executable once and cache it, so repeated
    kernel() calls skip jax re-tracing and NEFF compilation."""
    if "runner" in _CACHED:
        return _CACHED["runner"]

    import jax
    from jax.sharding import Mesh, NamedSharding, PartitionSpec
    from jax.experimental.shard_map import shard_map
    from concourse import bass2jax
    from concourse.bass2jax import _bass_exec_p, install_neuronx_cc_hook

    nc = _get_nc()
    install_neuronx_cc_hook()
    partition_name = nc.partition_id_tensor.name if nc.partition_id_tensor else None
    in_names, out_names, out_avals, zero_outs = [], [], [], []
    for alloc in nc.m.functions[0].allocations:
        if not isinstance(alloc, mybir.MemoryLocationSet):
            continue
        name = alloc.memorylocations[0].name
        if alloc.kind == "ExternalInput":
            if name != partition_name:
                in_names.append(name)
        elif alloc.kind == "ExternalOutput":
            out_names.append(name)
            shape = tuple(alloc.tensor_shape)
            dtype = mybir.dt.np(alloc.dtype)
            out_avals.append(jax.core.ShapedArray(shape, dtype))
            zero_outs.append(np.zeros(shape, dtype))
    n_params, n_outs = len(in_names), len(out_avals)
    all_names = in_names + out_names + ([partition_name] if partition_name else [])

    def _body(*args):
        operands = list(args)
        if partition_name is not None:
            operands.append(bass2jax.partition_id_tensor())
        outs = _bass_exec_p.bind(
            *operands,
            out_avals=tuple(out_avals),
            in_names=tuple(all_names),
            out_names=tuple(out_names),
            lowering_input_output_aliases=(),
            sim_require_finite=True,
            sim_require_nnan=True,
            nc=nc,
        )
        return tuple(outs)

    devices = jax.devices()[:8]
    mesh = Mesh(np.asarray(devices), ("core",))
    f = jax.jit(
        shard_map(
            _body, mesh=mesh,
            in_specs=(PartitionSpec("core"),) * (n_params + n_outs),
            out_specs=(PartitionSpec("core"),) * n_outs,
            check_rep=False,
        ),
        donate_argnums=tuple(range(n_params, n_params + n_outs)),
        keep_unused=True,
    )
    shard = NamedSharding(mesh, PartitionSpec("core"))
    state = {
        "f": f, "in_names": in_names, "out_names": out_names,
        "zero_outs": zero_outs, "shard": shard, "jax": jax, "last_outs": None,
    }
    _CACHED["runner"] = state
    return state


def kernel(encoder_input, attention_mask, Wq_w, Wq_b, Wk_w, Wk_b, Wv_w, Wv_b,
           Wo_w, Wo_b):
    del attention_mask  # dead input in the reference forward
    encoder_input = np.asarray(encoder_input, dtype=np.float32)
    Wq_w = np.asarray(Wq_w, dtype=np.float32)
    Wk_w = np.asarray(Wk_w, dtype=np.float32)
    Wv_w = np.asarray(Wv_w, dtype=np.float32)
    Wo_w = np.asarray(Wo_w, dtype=np.float32)
    Wq_b = np.asarray(Wq_b, dtype=np.float32)
    Wk_b = np.asarray(Wk_b, dtype=np.float32)
    Wv_b = np.asarray(Wv_b, dtype=np.float32)
    Wo_b = np.asarray(Wo_b, dtype=np.float32)

    r = _get_runner()
    jax = r["jax"]

    woT_f16 = {}
    in_maps = []
    for core in range(8):
        b, g = divmod(core, G)
        gs = slice(g * DL, (g + 1) * DL)
        if g not in woT_f16:
            woT_f16[g] = np.ascontiguousarray(Wo_w[:, gs].T).astype(np.float16)
        in_maps.append({
            "xT": np.ascontiguousarray(encoder_input[b].T),
            "wqT": np.ascontiguousarray(Wq_w[gs, :].T),
            "wkT": np.ascontiguousarray(Wk_w[gs, :].T),
            "wvT": np.ascontiguousarray(Wv_w[gs, :].T),
            "woT": woT_f16[g],
            "bq": np.ascontiguousarray(Wq_b[gs]),
            "bk": np.ascontiguousarray(Wk_b[gs]),
            "bv": Wv_b[gs].astype(np.float16).reshape(1, DL),
            "bo": (Wo_b if g == 0 else np.zeros_like(Wo_b))
                  .astype(np.float16).reshape(1, D),
        })

    concat_in = [
        jax.device_put(
            np.concatenate([in_maps[c][n] for c in range(8)], axis=0), r["shard"])
        for n in r["in_names"]
    ]
    outs = r["last_outs"]
    if outs is None:
        outs = [
            jax.device_put(
                np.zeros((8 * z.shape[0], *z.shape[1:]), z.dtype), r["shard"])
            for z in r["zero_outs"]
        ]
    outs = r["f"](*concat_in, *outs)
    np_outs = [np.asarray(o) for o in outs]
    # keep the returned device buffers to donate on the next call
    r["last_outs"] = list(outs)

    per_core = {}
    for i, nme in enumerate(r["out_names"]):
        full = np_outs[i].reshape(8, -1, *np_outs[i].shape[1:])
        per_core[nme] = full

    y = per_core["y"]
    out = np.empty((B, S, D), dtype=np.float32)
    for b in range(B):
        out[b] = y[G * b] + y[G * b + 1]
    return out
